# revision 49
# baseline (speedup 1.0000x reference)
"""DTGNN Trainium2 Bass kernel.

Single-core algorithm (graph is tiny: N=8, E=16), replicated across the 8
NeuronCores via SPMD; core 0's output is returned. All gather/scatter over
edge_index is done on-device with one-hot matmuls built by iota/is_equal.

v2: bf16 matmul operands throughout (4x PE throughput in the cost model),
attention-score reductions (h*a_s etc.) moved from DVE mult+reduce onto PE
via transposed-h matmuls against host-packed block-diagonal `a` matrices,
edge-MLP biases folded into the contraction as extra rows, final linear pair
folded on the host (no relu between them), deconv-pool scale/bias folded
into the selector matmul, act-func table preloaded during the DMA window.
"""
import numpy as np
from contextlib import ExitStack

import ml_dtypes

import concourse.bacc as bacc
import concourse.bass as bass
import concourse.tile as tile
import concourse.mybir as mybir
from concourse.bass_utils import run_bass_kernel_spmd

F32 = mybir.dt.float32
BF16 = mybir.dt.bfloat16
I32 = mybir.dt.int32
ALU = mybir.AluOpType
ACT = mybir.ActivationFunctionType
AXL = mybir.AxisListType

# ---------------------------------------------------------------------------
# column layouts of the packed DRAM inputs
# ---------------------------------------------------------------------------
_LA0 = [("w1T", 24), ("TPAD", 24), ("w2T", 3), ("M24T", 24), ("ident16", 16),
        ("c1b1row", 8), ("ones20", 20), ("c1b2c", 1), ("ones16", 16),
        ("XT", 32)]
_LA = [("mlpw1", 64), ("Wa2", 4), ("mlpw2x", 64), ("v2h", 1),
       ("mlpb1", 1), ("g1b8", 256), ("g2b8", 64), ("ones8b", 1)]
_LB1 = [("G1L", 1024), ("g2l", 128), ("eaT", 16), ("g1ae_w", 4), ("Was", 32),
        ("was510", 8), ("was511", 8), ("g1t0", 256), ("g1t1", 256)]
_LB2 = [("dst1", 640), ("dst2x", 640), ("w12T", 20), ("c2w2T", 192),
        ("c2w1T", 96)]
_LF = [("iota_row24", 8), ("ipackbits", 50), ("iota8", 1),
       ("b12", 1), ("eye4", 4), ("c2b1", 1)]


def _mkoff(lst):
    d, o = {}, 0
    for name, w in lst:
        d[name] = o
        o += w
    d["_W"] = o
    return d


_oA0, _oA, _oB1, _oB2, _oF = (_mkoff(_LA0), _mkoff(_LA), _mkoff(_LB1),
                              _mkoff(_LB2), _mkoff(_LF))


def _build_nc():
    nc = bacc.Bacc("TRN2", target_bir_lowering=False)

    SF = _oA0["_W"]              # f32 const block rides tA0, bitcast-packed
    SA = SF + 2 * _oF["_W"]
    Wb = SA + _oA["_W"] + _oB1["_W"] + _oB2["_W"]
    mb = nc.dram_tensor("mb", [128, Wb], BF16, kind="ExternalInput")
    out = nc.dram_tensor("out", [10, 64], F32, kind="ExternalOutput")
    SB1 = SA + _oA["_W"]
    SB2 = SB1 + _oB1["_W"]

    with tile.TileContext(nc) as tc, ExitStack() as ctx:
        def _go():
            ctx.enter_context(nc.allow_low_precision(reason="tol 2e-2; bf16 ok"))
            sb = ctx.enter_context(tc.tile_pool(name="sb", bufs=1))
            ps = ctx.enter_context(tc.tile_pool(name="ps", bufs=4, space="PSUM"))
            pst = ctx.enter_context(tc.tile_pool(name="pst", bufs=3, space="PSUM"))

            # dummy act: forces the act-func-table load to run during the
            # input-DMA window instead of before the first real activation
            dumb = sb.tile([1, 1], F32)
            nc.vector.memset(dumb[:], 0.0)
            nc.scalar.activation(dumb[:], dumb[:], ACT.Exp)
            # PE p-state warmup: back-to-back dummy matmuls so the tensor
            # engine is past its 3us ramp when the real matmuls arrive
            dscr = sb.tile([1, 192], F32)
            nc.vector.memset(dscr[:], 0.0)
            ps_warm = ps.tile([1, 192], F32, tag="ps")
            nc.tensor.matmul(ps_warm[:], dscr[:, 0:1], dscr[:], start=True, stop=False)
            nc.tensor.matmul(ps_warm[:], dscr[:, 0:1], dscr[:], start=False, stop=True)

            def pe_keepwarm(n):
                for _ in range(n):
                    pw = ps.tile([1, 128], F32, tag="ps", name="pw")
                    nc.tensor.matmul(pw[:], dscr[:, 0:1], dscr[:, 0:128],
                                     start=True, stop=True)

            def pe_keepwarm_small(n):
                for _ in range(n):
                    pw2 = ps.tile([1, 32], F32, tag="ps", name="pw2")
                    nc.tensor.matmul(pw2[:], dscr[:, 0:1], dscr[:, 0:32],
                                     start=True, stop=True)

            # -------------------------------------------------- input DMAs
            tA0 = sb.tile([128, SA], BF16)
            nc.sync.dma_start(tA0[:], mb[:, 0:SA])
            tB1 = sb.tile([128, _oB1["_W"]], BF16)
            nc.sync.dma_start(tB1[:], mb[:, SB1:SB2])
            tA = sb.tile([128, _oA["_W"]], BF16)
            nc.sync.dma_start(tA[:], mb[:, SA:SB1])
            tB2 = sb.tile([128, _oB2["_W"]], BF16)
            nc.sync.dma_start(tB2[:], mb[:, SB2:])

            def B(t, off, name, w, rows):
                return t[0:rows, off[name]:off[name] + w]

            w1T = B(tA0, _oA0, "w1T", 24, 8).rearrange("p (k n) -> p k n", k=3)
            TPAD = B(tA0, _oA0, "TPAD", 24, 8).rearrange("p (b n) -> p b n", b=2)
            w2T = B(tA0, _oA0, "w2T", 3, 10)
            M24T = B(tA0, _oA0, "M24T", 24, 16)
            ident16 = B(tA0, _oA0, "ident16", 16, 16)
            ident8 = ident16[0:8, 0:8]
            c1b1row = B(tA0, _oA0, "c1b1row", 8, 1)
            ones20 = B(tA0, _oA0, "ones20", 20, 1)
            c1b2c = B(tA0, _oA0, "c1b2c", 1, 1)
            ones16 = B(tA0, _oA0, "ones16", 16, 1)

            def Bf(name, w, rows, dt=F32):
                o = SF + 2 * _oF[name]
                return tA0[0:rows, o:o + 2 * w].bitcast(dt)

            XT = B(tA0, _oA0, "XT", 32, 128).rearrange("p (j n) -> p j n", j=4)
            mlpw1 = B(tA, _oA, "mlpw1", 64, 128)
            Wa2 = B(tA, _oA, "Wa2", 4, 128).rearrange("p (j n) -> p j n", j=2)
            mlpw2x = B(tA, _oA, "mlpw2x", 64, 65)
            v2h = B(tA, _oA, "v2h", 1, 64)
            mlpb1 = B(tA, _oA, "mlpb1", 1, 64)
            g1b8 = B(tA, _oA, "g1b8", 256, 8)
            g2b8 = B(tA, _oA, "g2b8", 64, 8)
            ones8b = B(tA, _oA, "ones8b", 1, 8)

            G1L = B(tB1, _oB1, "G1L", 1024, 128).rearrange("p (j c) -> p j c", j=4)
            g2l = B(tB1, _oB1, "g2l", 128, 128).rearrange("p (j c) -> p j c", j=2)
            eaT = B(tB1, _oB1, "eaT", 16, 128)
            g1ae_w = B(tB1, _oB1, "g1ae_w", 4, 128)
            Was = B(tB1, _oB1, "Was", 32, 128).rearrange("p (j n) -> p j n", j=4)
            was510 = B(tB1, _oB1, "was510", 8, 1)
            was511 = B(tB1, _oB1, "was511", 8, 1)
            g1t0 = B(tB1, _oB1, "g1t0", 256, 1)
            g1t1 = B(tB1, _oB1, "g1t1", 256, 1)

            dst1 = B(tB2, _oB2, "dst1", 640, 128)
            dst2x = B(tB2, _oB2, "dst2x", 640, 68)
            w12T = B(tB2, _oB2, "w12T", 20, 64).rearrange("p (l n) -> p l n", l=2)
            c2w2T = B(tB2, _oB2, "c2w2T", 192, 32).rearrange("p (k n) -> p k n", k=3)
            c2w1T = B(tB2, _oB2, "c2w1T", 96, 4).rearrange("p (k n) -> p k n", k=3)

            iota_row24 = Bf("iota_row24", 8, 24)
            iota8 = Bf("iota8", 1, 8)
            b12 = Bf("b12", 1, 10)
            eye4 = Bf("eye4", 4, 4)
            c2b1 = Bf("c2b1", 1, 32)

            # ------------------------------------------------------- CNN_1
            # (elementwise steps on DVE: shorter access latency than Act)
            ps_y1 = ps.tile([8, 2, 10], F32, tag="ps")
            nc.tensor.matmul(ps_y1[:], c1b1row,
                             ones20[:].rearrange("p (a b) -> p a b", a=2),
                             start=True, stop=False)
            for k in range(3):
                nc.tensor.matmul(ps_y1[:], w1T[:, k, :], TPAD[:, :, k:k + 10],
                                 start=False, stop=(k == 2))
            y1 = sb.tile([8, 2, 10], BF16)
            nc.vector.tensor_scalar(y1[:], ps_y1[:], 0.0, None, ALU.max)

            ps_z = ps.tile([10, 2, 8], BF16, tag="ps")
            nc.tensor.transpose(ps_z[:, 0, :], y1[:, 0, :], ident8)
            nc.tensor.transpose(ps_z[:, 1, :], y1[:, 1, :], ident8)
            zp = sb.tile([10, 2, 10], BF16)
            nc.vector.memset(zp[:], 0.0)
            nc.vector.tensor_copy(zp[:, :, 1:9], ps_z[:])

            ps_y2 = ps.tile([1, 16], F32, tag="ps")
            nc.tensor.matmul(ps_y2[:], c1b2c, ones16[:], start=True, stop=False)
            for k in range(3):
                nc.tensor.matmul(ps_y2[:], w2T[:, k:k + 1], zp[:, :, k:k + 8],
                                 start=False, stop=(k == 2))
            # torch .view scramble: x_[n, c] = flat[2n+c] -> xr01 col c*8+n
            xr01 = sb.tile([1, 16], BF16)  # cols 0:8 = feat 510, 8:16 = feat 511
            nc.vector.tensor_scalar(xr01[:].rearrange("p (c n) -> p n c", c=2),
                                    ps_y2[:].rearrange("p (n c) -> p n c", c=2),
                                    0.0, None, ALU.max)

            # ---------------------------------------------- one-hot matrices
            ti = Bf("ipackbits", 50, 24, I32)
            tif = sb.tile([24, 50], F32)
            nc.vector.tensor_copy(tif[:], ti)
            idx_f = tif[0:8, 0:48].rearrange("p (c e) -> p c e", c=2)
            dcol_f = tif[:, 48:49]

            PsrcT = sb.tile([8, 24], BF16)   # [n, e] = (src[e]==n)
            nc.vector.tensor_scalar(PsrcT[:], idx_f[:, 0, :], iota8, None, ALU.is_equal)
            PdstT = sb.tile([8, 24], BF16)   # [n, e] = (dst[e]==n)
            nc.vector.tensor_scalar(PdstT[:], idx_f[:, 1, :], iota8, None, ALU.is_equal)
            Pdst = sb.tile([24, 8], BF16)    # [e, n] = (dst[e]==n)
            nc.vector.tensor_scalar(Pdst[:], iota_row24, dcol_f, None, ALU.is_equal)

            # ------------------------------------------------------- GAT 1
            # attention scalars: asad = x @ (g1_lin @ asadcols) in cols 0:8,
            # ae16 = ea @ (g1_le @ aecols) in cols 8:12 — one tile, one copy
            ps_att = ps.tile([16, 12], F32, tag="ps")
            nc.tensor.matmul(ps_att[0:16, 8:12], eaT, g1ae_w, start=True, stop=True)
            for j in range(4):
                nc.tensor.matmul(ps_att[0:8, 0:8], XT[:, j, :], Was[:, j, :],
                                 start=(j == 0), stop=False)
            nc.tensor.matmul(ps_att[0:8, 0:8], xr01[:, 0:8], was510,
                             start=False, stop=False)
            nc.tensor.matmul(ps_att[0:8, 0:8], xr01[:, 8:16], was511,
                             start=False, stop=True)
            att = sb.tile([16, 12], BF16)
            nc.vector.tensor_copy(att[:], ps_att[:])
            asad = att[0:8, 0:8]
            ae16 = att[0:16, 8:12]

            ps_h = ps.tile([8, 256], F32, tag="ps")
            for j in range(4):
                nc.tensor.matmul(ps_h[:], XT[:, j, :], G1L[:, j, :],
                                 start=(j == 0), stop=False)
            nc.tensor.matmul(ps_h[:], xr01[:, 0:8], g1t0[:], start=False, stop=False)
            nc.tensor.matmul(ps_h[:], xr01[:, 8:16], g1t1[:], start=False, stop=True)
            hsb = sb.tile([8, 256], BF16)
            nc.vector.tensor_copy(hsb[:], ps_h[:])

            # alpha (pre-activation) = as[src] + ad[dst] + ae, all 24 edges
            ps_al = ps.tile([24, 4], F32, tag="ps")
            nc.tensor.matmul(ps_al[:], PsrcT[:], asad[:, 0:4], start=True, stop=False)
            nc.tensor.matmul(ps_al[:], PdstT[:], asad[:, 4:8], start=False, stop=False)
            nc.tensor.matmul(ps_al[:], M24T, ae16, start=False, stop=True)
            lr1 = sb.tile([24, 4], F32)
            nc.scalar.activation(lr1[:], ps_al[:], ACT.Prelu, alpha=0.2)
            ex24 = sb.tile([24, 4], BF16)
            nc.scalar.activation(ex24[:], lr1[:], ACT.Exp)

            ps_sg = ps.tile([24, 256], F32, tag="ps")
            nc.tensor.matmul(ps_sg[:], PsrcT[:], hsb[:], start=True, stop=True)
            ps_den = ps.tile([8, 4], F32, tag="ps")
            nc.tensor.matmul(ps_den[:], Pdst[:], ex24[:], start=True, stop=True)
            rden = sb.tile([8, 4], F32)
            nc.vector.reciprocal(rden[:], ps_den[:])

            wh = sb.tile([24, 256], BF16)
            nc.vector.tensor_tensor(wh[:].rearrange("p (h c) -> p h c", h=4),
                                    ps_sg[:].rearrange("p (h c) -> p h c", h=4),
                                    ex24[:].broadcast_to([24, 4, 64]), ALU.mult)
            ps_num = ps.tile([8, 256], F32, tag="ps")
            nc.tensor.matmul(ps_num[:], Pdst[:], wh[:], start=True, stop=True)

            x1t = sb.tile([8, 256], BF16)
            nc.vector.tensor_tensor(x1t[:].rearrange("p (h c) -> p h c", h=4),
                                    ps_num[:].rearrange("p (h c) -> p h c", h=4),
                                    rden[:].broadcast_to([8, 4, 64]), ALU.mult)
            x1b = sb.tile([8, 256], BF16)
            nc.vector.tensor_tensor(x1b[:], x1t[:], g1b8, ALU.add)

            # ---------------------------- edge MLP (transposed, dual copies)
            ps_m1 = ps.tile([64, 16], F32, tag="ps")
            nc.tensor.matmul(ps_m1[:], mlpw1, eaT, start=True, stop=True)
            r1Tx = sb.tile([65, 16], BF16)
            nc.vector.memset(r1Tx[64:65, :], 1.0)   # bias row for mlpw2x
            nc.scalar.activation(r1Tx[0:64, :], ps_m1[:], ACT.Relu, bias=mlpb1)
            ps_m2 = ps.tile([128, 16], F32, tag="ps")
            nc.tensor.matmul(ps_m2[0:64, :], mlpw2x, r1Tx[:], start=True, stop=True)
            nc.tensor.matmul(ps_m2[64:128, :], mlpw2x, r1Tx[:], start=True, stop=True)
            eaNT = sb.tile([128, 16], BF16)
            nc.scalar.copy(eaNT[:], ps_m2[:])

            # ------------------------------------------------------- GAT 2
            ps_xt = ps.tile([128, 2, 8], BF16, tag="ps")
            nc.tensor.transpose(ps_xt[:, 0, :], x1b[:, 0:128], ident8)
            nc.tensor.transpose(ps_xt[:, 1, :], x1b[:, 128:256], ident8)
            pe_keepwarm_small(2)
            x1T = sb.tile([128, 2, 8], BF16)
            nc.scalar.activation(x1T[:], ps_xt[:], ACT.Relu)

            ps_h2 = ps.tile([8, 64], F32, tag="ps")
            for j in range(2):
                nc.tensor.matmul(ps_h2[:], x1T[:, j, :], g2l[:, j, :],
                                 start=(j == 0), stop=(j == 1))
            ps_att2 = ps.tile([16, 3], F32, tag="ps")
            nc.tensor.matmul(ps_att2[0:16, 2:3], eaNT[0:64, :], v2h[:],
                             start=True, stop=True)
            for j in range(2):
                nc.tensor.matmul(ps_att2[0:8, 0:2], x1T[:, j, :], Wa2[:, j, :],
                                 start=(j == 0), stop=(j == 1))
            pe_keepwarm_small(4)
            att2 = sb.tile([16, 3], BF16)
            nc.vector.tensor_copy(att2[:], ps_att2[:])
            a2 = att2[0:8, 0:2]
            e16 = att2[0:16, 2:3]
            h2sb = sb.tile([8, 64], BF16)
            nc.vector.tensor_copy(h2sb[:], ps_h2[:])

            ps_al2 = ps.tile([24, 1], F32, tag="ps")
            nc.tensor.matmul(ps_al2[:], PsrcT[:], a2[:, 0:1], start=True, stop=False)
            nc.tensor.matmul(ps_al2[:], PdstT[:], a2[:, 1:2], start=False, stop=False)
            nc.tensor.matmul(ps_al2[:], M24T, e16, start=False, stop=True)
            lr2 = sb.tile([24, 1], F32)
            nc.scalar.activation(lr2[:], ps_al2[:], ACT.Prelu, alpha=0.2)
            ex2 = sb.tile([24, 1], F32)
            nc.scalar.activation(ex2[:], lr2[:], ACT.Exp)
            ex2b = sb.tile([24, 1], BF16)
            nc.vector.tensor_copy(ex2b[:], ex2[:])

            ps_sg2 = ps.tile([24, 64], F32, tag="ps")
            nc.tensor.matmul(ps_sg2[:], PsrcT[:], h2sb[:], start=True, stop=True)
            ps_den2 = ps.tile([8, 1], F32, tag="ps")
            nc.tensor.matmul(ps_den2[:], Pdst[:], ex2b[:], start=True, stop=True)
            rden2 = sb.tile([8, 1], F32)
            nc.vector.reciprocal(rden2[:], ps_den2[:])

            wh2 = sb.tile([24, 64], BF16)
            nc.vector.tensor_scalar(wh2[:], ps_sg2[:], ex2[:], None, ALU.mult)
            ps_num2 = ps.tile([8, 64], F32, tag="ps")
            nc.tensor.matmul(ps_num2[:], Pdst[:], wh2[:], start=True, stop=True)

            x2b = sb.tile([8, 64], BF16)
            nc.vector.scalar_tensor_tensor(x2b[:], ps_num2[:], rden2[:], g2b8,
                                           ALU.mult, ALU.add)
            x2 = sb.tile([8, 64], BF16)
            nc.vector.tensor_scalar(x2[:], x2b[:], 0.0, None, ALU.max)

            # ------------------- deconv pool rows via block-diagonal selector
            pe_keepwarm(3)
            ps_xm = ps.tile([64, 1], F32, tag="ps")
            nc.tensor.matmul(ps_xm[:], x2[:], ones8b, start=True, stop=True)

            sel = sb.tile([128, 4], BF16)
            nc.vector.memset(sel[:], 0.0)
            nc.scalar.copy(sel[0:64, 0:1], ps_xm[:])
            eaview = eaNT[:].rearrange("p (n two) -> p n two", two=2)
            nc.vector.tensor_reduce(sel[64:128, 2:3], eaview[64:128, :, 0],
                                    axis=AXL.X, op=ALU.add)
            # sel2 rows 64:68 select the 4 bias rows appended to dst2x
            sel2 = sb.tile([68, 4], BF16)
            nc.vector.memset(sel2[:], 0.0)
            nc.vector.tensor_copy(sel2[64:68, :], eye4)
            nc.vector.tensor_reduce(sel2[0:64, 3:4], eaview[0:64, :, 1],
                                    axis=AXL.X, op=ALU.add)

            cT = sb.tile([4, 640], BF16)
            ps_cTa = pst.tile([4, 320], F32, tag="pst")
            nc.tensor.matmul(ps_cTa[:], sel[:], dst1[:, 0:320], start=True, stop=False)
            nc.tensor.matmul(ps_cTa[:], sel2[:], dst2x[:, 0:320], start=False, stop=True)
            nc.vector.tensor_copy(cT[:, 0:320], ps_cTa[:])
            ps_cTb = pst.tile([4, 320], F32, tag="pst")
            nc.tensor.matmul(ps_cTb[:], sel[:], dst1[:, 320:640], start=True, stop=False)
            nc.tensor.matmul(ps_cTb[:], sel2[:], dst2x[:, 320:640], start=False, stop=True)
            nc.scalar.copy(cT[:, 320:640], ps_cTb[:])

            # ------------------------------------------------------- CNN_2
            # conv1 split by cell halves so each half starts as soon as its
            # cT columns land
            cTv = cT[:].rearrange("p (b l) -> p b l", b=64)
            ps_c1 = pst.tile([32, 64, 8], F32, tag="pst")
            for k in range(3):
                nc.tensor.matmul(ps_c1[:, 0:32, :], c2w1T[:, k, :],
                                 cTv[:, 0:32, k:k + 8],
                                 start=(k == 0), stop=(k == 2))
            for k in range(3):
                nc.tensor.matmul(ps_c1[:, 32:64, :], c2w1T[:, k, :],
                                 cTv[:, 32:64, k:k + 8],
                                 start=(k == 0), stop=(k == 2))
            # maxpool -> bias -> conv2 -> folded linear, pipelined in cell
            # halves so each stage starts when its half of PSUM lands.
            # l1+l2 are linear-linear (no relu between): folded on host into
            # W12 [128,10]; c2b2's contribution is folded into b12.
            pc1v = ps_c1[:].rearrange("p b (l two) -> p b l two", two=2)
            mp = sb.tile([32, 64, 4], BF16)
            ps_c2 = pst.tile([64, 64, 2], F32, tag="pst")
            y2c = sb.tile([64, 64, 2], BF16)
            ps_l2 = pst.tile([10, 64], F32, tag="pst")
            for hi, (h0, h1) in enumerate(((0, 32), (32, 64))):
                nc.vector.tensor_reduce(mp[:, h0:h1, :], pc1v[:, h0:h1, :, :],
                                        axis=AXL.X, op=ALU.max)
                for k in range(3):
                    nc.tensor.matmul(ps_c2[:, h0:h1, :], c2w2T[:, k, :],
                                     mp[:, h0:h1, k:k + 2],
                                     start=(k == 0), stop=(k == 2))
                nc.vector.tensor_copy(y2c[:, h0:h1, :], ps_c2[:, h0:h1, :])
                for l in range(2):
                    nc.tensor.matmul(ps_l2[:, h0:h1], w12T[:, l, :],
                                     y2c[:, h0:h1, l],
                                     start=(l == 0), stop=(l == 1))
            o10 = sb.tile([10, 64], F32)
            nc.vector.tensor_scalar(o10[:, 0:32], ps_l2[:, 0:32], b12, 0.0,
                                    ALU.add, ALU.max)
            nc.vector.tensor_scalar(o10[:, 32:64], ps_l2[:, 32:64], b12, 0.0,
                                    ALU.add, ALU.max)
            nc.sync.dma_start(out[:], o10[:])

        _go()
    nc.finalize()
    return nc


_NC = None


def _get_nc():
    global _NC
    if _NC is None:
        _NC = _build_nc()
    return _NC


def _pack_inputs(x_feat, x_feat_tmp, edge_attr, c1w1, c1b1, c1w2, c1b2,
                 g1_lin, g1_as, g1_ad, g1_le, g1_ae, g1_b,
                 g2_lin, g2_as, g2_ad, g2_le, g2_ae, g2_b,
                 mlp_w1, mlp_b1, mlp_w2, mlp_b2,
                 d1w, d1b, d2w, d2b, d3w, d3b,
                 c2w1, c2b1, c2w2, c2b2, c2l1w, c2l1b, c2l2w, c2l2b,
                 edge_index):
    f = np.float32

    def fill(shape, off, blocks):
        arr = np.zeros(shape, dtype=f)
        for name, a in blocks.items():
            a = np.asarray(a, dtype=f)
            arr[0:a.shape[0], off[name]:off[name] + a.shape[1]] = a
        return arr

    xfT = np.zeros((512, 8), dtype=f)
    xfT[0:510] = x_feat.T

    tpad = np.zeros((8, 2, 12), dtype=f)
    for i in range(8):
        r = 1 if i % 2 == 0 else 5
        for b in range(2):
            tpad[i, b, 1:11] = x_feat_tmp[r, b * 4 + i // 2]

    m24mat = np.zeros((16, 24), dtype=f)
    m24mat[:, 0:16] = np.eye(16, dtype=f)
    m24mat[:, 16:24] = 1.0 / 16.0

    # host-folded weight-only attention chains
    asadcols = np.zeros((256, 8), dtype=f)
    aecols = np.zeros((256, 4), dtype=f)
    for h in range(4):
        asadcols[h * 64:(h + 1) * 64, h] = g1_as[h]
        asadcols[h * 64:(h + 1) * 64, 4 + h] = g1_ad[h]
        aecols[h * 64:(h + 1) * 64, h] = g1_ae[h]
    Was = np.asarray(g1_lin, dtype=np.float64) @ asadcols.astype(np.float64)  # [512, 8]
    g1ae_w = np.asarray(g1_le, dtype=np.float64) @ aecols.astype(np.float64)  # [128, 4]
    Wa2 = (np.asarray(g2_lin, dtype=np.float64)
           @ np.stack([g2_as[0], g2_ad[0]], 1).astype(np.float64))            # [256, 2]
    v2harr = np.asarray(g2_le, dtype=np.float64) @ np.asarray(g2_ae[0], dtype=np.float64)

    mA0 = fill((128, _oA0["_W"]), _oA0, {
        "w1T": c1w1.transpose(1, 2, 0).reshape(8, 24),
        "TPAD": tpad.reshape(8, 24),
        "w2T": c1w2.transpose(1, 2, 0).reshape(10, 3),
        "M24T": m24mat,
        "ident16": np.eye(16, dtype=f),
        "XT": xfT.reshape(4, 128, 8).transpose(1, 0, 2).reshape(128, 32),
        "c1b1row": c1b1.reshape(1, 8),
        "ones20": np.ones((1, 20), dtype=f),
        "c1b2c": c1b2.reshape(1, 1),
        "ones16": np.ones((1, 16), dtype=f),
    })
    mA = fill((128, _oA["_W"]), _oA, {
        "mlpw1": mlp_w1,
        "Wa2": Wa2.astype(f).reshape(2, 128, 2).transpose(1, 0, 2).reshape(128, 4),
        "mlpw2x": np.concatenate([np.asarray(mlp_w2, dtype=f),
                                  np.asarray(mlp_b2, dtype=f).reshape(1, 64)], 0),
        "v2h": v2harr.astype(f).reshape(64, 1),
        "mlpb1": mlp_b1.reshape(64, 1),
        "g1b8": np.broadcast_to(g1_b.reshape(1, 256), (8, 256)),
        "g2b8": np.broadcast_to(g2_b.reshape(1, 64), (8, 64)),
        "ones8b": np.full((8, 1), 0.125, dtype=f),
    })

    mB1 = fill((128, _oB1["_W"]), _oB1, {
        "G1L": g1_lin.reshape(4, 128, 256).transpose(1, 0, 2).reshape(128, 1024),
        "g2l": g2_lin.reshape(2, 128, 64).transpose(1, 0, 2).reshape(128, 128),
        "eaT": edge_attr.T,
        "g1ae_w": g1ae_w.astype(f),
        "Was": Was[0:512].astype(f).reshape(4, 128, 8).transpose(1, 0, 2).reshape(128, 32),
        "was510": Was[510:511].astype(f),
        "was511": Was[511:512].astype(f),
        "g1t0": g1_lin[510:511],
        "g1t1": g1_lin[511:512],
    })

    # bias4 rows (unscaled; 0.125 pool scales are folded into d2w/d3w)
    b4 = np.zeros((4, 640), dtype=f)
    b4[0] = np.repeat(d1b, 10)
    b4[1] = np.asarray(x_feat_tmp, dtype=f).reshape(640)   # inf rides the bias row
    b4[2] = np.repeat(d2b, 10)
    b4[3] = np.repeat(d3b, 10)
    dst2x = np.concatenate([np.asarray(d3w, dtype=f).reshape(64, 640) * 0.125,
                            b4], 0)

    # final linear pair (no relu between) folded: c -> c @ W12 + b12f
    W12 = (np.asarray(c2l1w, dtype=np.float64) @ np.asarray(c2l2w, dtype=np.float64))
    cw2b = np.einsum('ock,c->o', np.asarray(c2w2, dtype=np.float64),
                     np.asarray(c2b1, dtype=np.float64))   # conv2(c2b1 const)
    b12f = (np.asarray(c2l1b, dtype=np.float64) @ np.asarray(c2l2w, dtype=np.float64)
            + np.asarray(c2l2b, dtype=np.float64)
            + np.repeat(np.asarray(c2b2, dtype=np.float64), 2) @ W12
            + np.repeat(cw2b, 2) @ W12)
    W12 = W12.astype(f)

    mB2 = fill((128, _oB2["_W"]), _oB2, {
        "dst1": np.concatenate([d1w.reshape(64, 640),
                                d2w.reshape(64, 640) * 0.125], 0),
        "dst2x": dst2x,
        "w12T": np.concatenate([W12[0::2], W12[1::2]], 1),
        "c2w2T": c2w2.transpose(1, 2, 0).reshape(32, 192),
        "c2w1T": c2w1.transpose(1, 2, 0).reshape(4, 96),
    })

    mfarr = fill((32, _oF["_W"]), _oF, {
        "iota_row24": np.broadcast_to(np.arange(8, dtype=f), (24, 8)),
        "iota8": np.arange(8, dtype=f).reshape(8, 1),
        "b12": b12f.astype(f).reshape(10, 1),
        "eye4": np.eye(4, dtype=f),
        "c2b1": c2b1.reshape(32, 1),
    })
    ipack = np.zeros((24, 50), dtype=np.int32)
    blk = np.zeros((8, 2, 24), dtype=np.int32)
    blk[:, :, 0:16] = edge_index[None, :, :]
    blk[:, :, 16:24] = np.arange(8, dtype=np.int32)[None, None, :]
    ipack[0:8, 0:48] = blk.reshape(8, 48)
    ipack[0:16, 48] = edge_index[1]
    ipack[16:24, 48] = np.arange(8, dtype=np.int32)
    mfarr[0:24, _oF["ipackbits"]:_oF["ipackbits"] + 50] = ipack.view(np.float32)

    # f32 block bit-packed into bf16 columns right after mA0
    fblk = np.zeros((128, 2 * _oF["_W"]), dtype=ml_dtypes.bfloat16)
    fblk[0:32] = mfarr.view(np.uint16).view(ml_dtypes.bfloat16)
    mb = np.concatenate(
        [mA0.astype(ml_dtypes.bfloat16), fblk] +
        [m.astype(ml_dtypes.bfloat16) for m in (mA, mB1, mB2)], 1)
    return mb


def _make_ins(inputs):
    mb = _pack_inputs(**inputs)
    return {"mb": mb}


def kernel(**inputs):
    inputs = {k: np.ascontiguousarray(v) for k, v in inputs.items()}
    ins = _make_ins(inputs)
    nc = _get_nc()
    res = run_bass_kernel_spmd(nc, [ins] * 8, core_ids=list(range(8)))
    return np.ascontiguousarray(res.results[0]["out"].T).reshape(8, 8, 10)


# revision 50
# speedup vs baseline: 1.0156x; 1.0156x over previous
"""DTGNN Trainium2 Bass kernel.

Single-core algorithm (graph is tiny: N=8, E=16), replicated across the 8
NeuronCores via SPMD; core 0's output is returned. All gather/scatter over
edge_index is done on-device with one-hot matmuls built by iota/is_equal.

Optimized for the TimelineSim cost model (latency-bound regime): bf16 matmul
operands throughout; every weights-only linear subchain folded on the host
(attention-score projections, final linear pair, deconv-pool scales/biases,
conv2/c2b1 constant); CNN/MLP biases folded into matmul contractions as extra
rank-1 terms; act-func table preloaded and PE p-state warmed during the DMA
window; f32 constants bit-packed into the bf16 weight tensor (widening
bitcast) so one first DMA unblocks the whole front of the graph; CNN_2 tail
pipelined in cell halves across PE/DVE/Act.
"""
import numpy as np
from contextlib import ExitStack

import ml_dtypes

import concourse.bacc as bacc
import concourse.bass as bass
import concourse.tile as tile
import concourse.mybir as mybir
from concourse.bass_utils import run_bass_kernel_spmd

F32 = mybir.dt.float32
BF16 = mybir.dt.bfloat16
I32 = mybir.dt.int32
ALU = mybir.AluOpType
ACT = mybir.ActivationFunctionType
AXL = mybir.AxisListType

# ---------------------------------------------------------------------------
# column layouts of the packed DRAM inputs
# ---------------------------------------------------------------------------
_LA0 = [("w1T", 24), ("TPAD", 24), ("w2T", 3), ("M24T", 24), ("ident16", 16),
        ("c1b1row", 8), ("ones20", 20), ("c1b2c", 1), ("ones16", 16),
        ("XT", 32)]
_LA = [("mlpw1", 64), ("Wa2", 4), ("mlpw2x", 64), ("v2h", 1),
       ("mlpb1", 1), ("g1b8", 256), ("g2b8", 64), ("ones8b", 1)]
_LB1 = [("G1L", 1024), ("g2l", 128), ("eaT", 16), ("g1ae_w", 4), ("Was", 32),
        ("was510", 8), ("was511", 8), ("g1t0", 256), ("g1t1", 256)]
_LB2 = [("dst1", 640), ("dst2x", 640), ("w12T", 20), ("c2w2T", 192),
        ("c2w1T", 96)]
_LF = [("iota_row24", 8), ("ipackbits", 50), ("iota8", 1),
       ("b12", 1), ("eye4", 4), ("c2b1", 1)]


def _mkoff(lst):
    d, o = {}, 0
    for name, w in lst:
        d[name] = o
        o += w
    d["_W"] = o
    return d


_oA0, _oA, _oB1, _oB2, _oF = (_mkoff(_LA0), _mkoff(_LA), _mkoff(_LB1),
                              _mkoff(_LB2), _mkoff(_LF))


def _build_nc():
    nc = bacc.Bacc("TRN2", target_bir_lowering=False)

    SF = _oA0["_W"]              # f32 const block rides tA0, bitcast-packed
    SA = SF + 2 * _oF["_W"]
    Wb = SA + _oA["_W"] + _oB1["_W"] + _oB2["_W"]
    mb = nc.dram_tensor("mb", [128, Wb], BF16, kind="ExternalInput")
    out = nc.dram_tensor("out", [10, 64], F32, kind="ExternalOutput")
    SB1 = SA + _oA["_W"]
    SB2 = SB1 + _oB1["_W"]

    with tile.TileContext(nc) as tc, ExitStack() as ctx:
        def _go():
            ctx.enter_context(nc.allow_low_precision(reason="tol 2e-2; bf16 ok"))
            sb = ctx.enter_context(tc.tile_pool(name="sb", bufs=1))
            ps = ctx.enter_context(tc.tile_pool(name="ps", bufs=4, space="PSUM"))
            pst = ctx.enter_context(tc.tile_pool(name="pst", bufs=3, space="PSUM"))

            # dummy act: forces the act-func-table load to run during the
            # input-DMA window instead of before the first real activation
            dumb = sb.tile([1, 1], F32)
            nc.vector.memset(dumb[:], 0.0)
            nc.scalar.activation(dumb[:], dumb[:], ACT.Exp)
            # PE p-state warmup: back-to-back dummy matmuls so the tensor
            # engine is past its 3us ramp when the real matmuls arrive
            dscr = sb.tile([1, 192], F32)
            nc.vector.memset(dscr[:], 0.0)
            ps_warm = ps.tile([1, 192], F32, tag="ps")
            nc.tensor.matmul(ps_warm[:], dscr[:, 0:1], dscr[:], start=True, stop=False)
            nc.tensor.matmul(ps_warm[:], dscr[:, 0:1], dscr[:], start=False, stop=True)

            def pe_keepwarm(n):
                for _ in range(n):
                    pw = ps.tile([1, 128], F32, tag="ps", name="pw")
                    nc.tensor.matmul(pw[:], dscr[:, 0:1], dscr[:, 0:128],
                                     start=True, stop=True)

            def pe_keepwarm_small(n):
                for _ in range(n):
                    pw2 = ps.tile([1, 32], F32, tag="ps", name="pw2")
                    nc.tensor.matmul(pw2[:], dscr[:, 0:1], dscr[:, 0:32],
                                     start=True, stop=True)

            # -------------------------------------------------- input DMAs
            tA0 = sb.tile([128, SA], BF16)
            nc.sync.dma_start(tA0[:], mb[:, 0:SA])
            tB1 = sb.tile([128, _oB1["_W"]], BF16)
            nc.sync.dma_start(tB1[:], mb[:, SB1:SB2])
            tA = sb.tile([128, _oA["_W"]], BF16)
            nc.sync.dma_start(tA[:], mb[:, SA:SB1])
            tB2 = sb.tile([128, _oB2["_W"]], BF16)
            nc.sync.dma_start(tB2[:], mb[:, SB2:])

            def B(t, off, name, w, rows):
                return t[0:rows, off[name]:off[name] + w]

            w1T = B(tA0, _oA0, "w1T", 24, 8).rearrange("p (k n) -> p k n", k=3)
            TPAD = B(tA0, _oA0, "TPAD", 24, 8).rearrange("p (b n) -> p b n", b=2)
            w2T = B(tA0, _oA0, "w2T", 3, 10)
            M24T = B(tA0, _oA0, "M24T", 24, 16)
            ident16 = B(tA0, _oA0, "ident16", 16, 16)
            ident8 = ident16[0:8, 0:8]
            c1b1row = B(tA0, _oA0, "c1b1row", 8, 1)
            ones20 = B(tA0, _oA0, "ones20", 20, 1)
            c1b2c = B(tA0, _oA0, "c1b2c", 1, 1)
            ones16 = B(tA0, _oA0, "ones16", 16, 1)

            def Bf(name, w, rows, dt=F32):
                o = SF + 2 * _oF[name]
                return tA0[0:rows, o:o + 2 * w].bitcast(dt)

            XT = B(tA0, _oA0, "XT", 32, 128).rearrange("p (j n) -> p j n", j=4)
            mlpw1 = B(tA, _oA, "mlpw1", 64, 128)
            Wa2 = B(tA, _oA, "Wa2", 4, 128).rearrange("p (j n) -> p j n", j=2)
            mlpw2x = B(tA, _oA, "mlpw2x", 64, 65)
            v2h = B(tA, _oA, "v2h", 1, 64)
            mlpb1 = B(tA, _oA, "mlpb1", 1, 64)
            g1b8 = B(tA, _oA, "g1b8", 256, 8)
            g2b8 = B(tA, _oA, "g2b8", 64, 8)
            ones8b = B(tA, _oA, "ones8b", 1, 8)

            G1L = B(tB1, _oB1, "G1L", 1024, 128).rearrange("p (j c) -> p j c", j=4)
            g2l = B(tB1, _oB1, "g2l", 128, 128).rearrange("p (j c) -> p j c", j=2)
            eaT = B(tB1, _oB1, "eaT", 16, 128)
            g1ae_w = B(tB1, _oB1, "g1ae_w", 4, 128)
            Was = B(tB1, _oB1, "Was", 32, 128).rearrange("p (j n) -> p j n", j=4)
            was510 = B(tB1, _oB1, "was510", 8, 1)
            was511 = B(tB1, _oB1, "was511", 8, 1)
            g1t0 = B(tB1, _oB1, "g1t0", 256, 1)
            g1t1 = B(tB1, _oB1, "g1t1", 256, 1)

            dst1 = B(tB2, _oB2, "dst1", 640, 128)
            dst2x = B(tB2, _oB2, "dst2x", 640, 68)
            w12T = B(tB2, _oB2, "w12T", 20, 64).rearrange("p (l n) -> p l n", l=2)
            c2w2T = B(tB2, _oB2, "c2w2T", 192, 32).rearrange("p (k n) -> p k n", k=3)
            c2w1T = B(tB2, _oB2, "c2w1T", 96, 4).rearrange("p (k n) -> p k n", k=3)

            iota_row24 = Bf("iota_row24", 8, 24)
            iota8 = Bf("iota8", 1, 8)
            b12 = Bf("b12", 1, 10)
            eye4 = Bf("eye4", 4, 4)
            c2b1 = Bf("c2b1", 1, 32)

            # ------------------------------------------------------- CNN_1
            # (elementwise steps on DVE: shorter access latency than Act)
            ps_y1 = ps.tile([8, 2, 10], F32, tag="ps")
            nc.tensor.matmul(ps_y1[:], c1b1row,
                             ones20[:].rearrange("p (a b) -> p a b", a=2),
                             start=True, stop=False)
            for k in range(3):
                nc.tensor.matmul(ps_y1[:], w1T[:, k, :], TPAD[:, :, k:k + 10],
                                 start=False, stop=(k == 2))
            y1 = sb.tile([8, 2, 10], BF16)
            nc.vector.tensor_scalar(y1[:], ps_y1[:], 0.0, None, ALU.max)

            ps_z = ps.tile([10, 2, 8], BF16, tag="ps")
            nc.tensor.transpose(ps_z[:, 0, :], y1[:, 0, :], ident8)
            nc.tensor.transpose(ps_z[:, 1, :], y1[:, 1, :], ident8)
            zp = sb.tile([10, 2, 10], BF16)
            nc.vector.memset(zp[:], 0.0)
            nc.vector.tensor_copy(zp[:, :, 1:9], ps_z[:])

            ps_y2 = ps.tile([1, 16], F32, tag="ps")
            nc.tensor.matmul(ps_y2[:], c1b2c, ones16[:], start=True, stop=False)
            for k in range(3):
                nc.tensor.matmul(ps_y2[:], w2T[:, k:k + 1], zp[:, :, k:k + 8],
                                 start=False, stop=(k == 2))
            # torch .view scramble: x_[n, c] = flat[2n+c] -> xr01 col c*8+n
            xr01 = sb.tile([1, 16], BF16)  # cols 0:8 = feat 510, 8:16 = feat 511
            nc.vector.tensor_scalar(xr01[:].rearrange("p (c n) -> p n c", c=2),
                                    ps_y2[:].rearrange("p (n c) -> p n c", c=2),
                                    0.0, None, ALU.max)

            # ---------------------------------------------- one-hot matrices
            ti = Bf("ipackbits", 50, 24, I32)
            tif = sb.tile([24, 50], F32)
            nc.vector.tensor_copy(tif[:], ti)
            idx_f = tif[0:8, 0:48].rearrange("p (c e) -> p c e", c=2)
            dcol_f = tif[:, 48:49]

            PsrcT = sb.tile([8, 24], BF16)   # [n, e] = (src[e]==n)
            nc.vector.tensor_scalar(PsrcT[:], idx_f[:, 0, :], iota8, None, ALU.is_equal)
            PdstT = sb.tile([8, 24], BF16)   # [n, e] = (dst[e]==n)
            nc.vector.tensor_scalar(PdstT[:], idx_f[:, 1, :], iota8, None, ALU.is_equal)
            Pdst = sb.tile([24, 8], BF16)    # [e, n] = (dst[e]==n)
            nc.vector.tensor_scalar(Pdst[:], iota_row24, dcol_f, None, ALU.is_equal)

            # ------------------------------------------------------- GAT 1
            # attention scalars: asad = x @ (g1_lin @ asadcols) in cols 0:8,
            # ae16 = ea @ (g1_le @ aecols) in cols 8:12 — one tile, one copy
            ps_att = ps.tile([16, 12], F32, tag="ps")
            nc.tensor.matmul(ps_att[0:16, 8:12], eaT, g1ae_w, start=True, stop=True)
            for j in range(4):
                nc.tensor.matmul(ps_att[0:8, 0:8], XT[:, j, :], Was[:, j, :],
                                 start=(j == 0), stop=False)
            nc.tensor.matmul(ps_att[0:8, 0:8], xr01[:, 0:8], was510,
                             start=False, stop=False)
            nc.tensor.matmul(ps_att[0:8, 0:8], xr01[:, 8:16], was511,
                             start=False, stop=True)
            att = sb.tile([16, 12], BF16)
            nc.vector.tensor_copy(att[:], ps_att[:])
            asad = att[0:8, 0:8]
            ae16 = att[0:16, 8:12]

            ps_h = ps.tile([8, 256], F32, tag="ps")
            for j in range(4):
                nc.tensor.matmul(ps_h[:], XT[:, j, :], G1L[:, j, :],
                                 start=(j == 0), stop=False)
            nc.tensor.matmul(ps_h[:], xr01[:, 0:8], g1t0[:], start=False, stop=False)
            nc.tensor.matmul(ps_h[:], xr01[:, 8:16], g1t1[:], start=False, stop=True)
            hsb = sb.tile([8, 256], BF16)
            nc.vector.tensor_copy(hsb[:], ps_h[:])

            # alpha (pre-activation) = as[src] + ad[dst] + ae, all 24 edges
            ps_al = ps.tile([24, 4], F32, tag="ps")
            nc.tensor.matmul(ps_al[:], PsrcT[:], asad[:, 0:4], start=True, stop=False)
            nc.tensor.matmul(ps_al[:], PdstT[:], asad[:, 4:8], start=False, stop=False)
            nc.tensor.matmul(ps_al[:], M24T, ae16, start=False, stop=True)
            lr1 = sb.tile([24, 4], F32)
            nc.scalar.activation(lr1[:], ps_al[:], ACT.Prelu, alpha=0.2)
            ex24 = sb.tile([24, 4], BF16)
            nc.scalar.activation(ex24[:], lr1[:], ACT.Exp)

            ps_sg = ps.tile([24, 256], F32, tag="ps")
            nc.tensor.matmul(ps_sg[:], PsrcT[:], hsb[:], start=True, stop=True)
            ps_den = ps.tile([8, 4], F32, tag="ps")
            nc.tensor.matmul(ps_den[:], Pdst[:], ex24[:], start=True, stop=True)
            rden = sb.tile([8, 4], F32)
            nc.vector.reciprocal(rden[:], ps_den[:])

            wh = sb.tile([24, 256], BF16)
            nc.vector.tensor_tensor(wh[:].rearrange("p (h c) -> p h c", h=4),
                                    ps_sg[:].rearrange("p (h c) -> p h c", h=4),
                                    ex24[:].broadcast_to([24, 4, 64]), ALU.mult)
            ps_num = ps.tile([8, 256], F32, tag="ps")
            nc.tensor.matmul(ps_num[:], Pdst[:], wh[:], start=True, stop=True)

            x1t = sb.tile([8, 256], BF16)
            nc.vector.tensor_tensor(x1t[:].rearrange("p (h c) -> p h c", h=4),
                                    ps_num[:].rearrange("p (h c) -> p h c", h=4),
                                    rden[:].broadcast_to([8, 4, 64]), ALU.mult)
            x1b = sb.tile([8, 256], BF16)
            nc.vector.tensor_tensor(x1b[:], x1t[:], g1b8, ALU.add)

            # ---------------------------- edge MLP (transposed, dual copies)
            ps_m1 = ps.tile([64, 16], F32, tag="ps")
            nc.tensor.matmul(ps_m1[:], mlpw1, eaT, start=True, stop=True)
            r1Tx = sb.tile([65, 16], BF16)
            nc.vector.memset(r1Tx[64:65, :], 1.0)   # bias row for mlpw2x
            nc.scalar.activation(r1Tx[0:64, :], ps_m1[:], ACT.Relu, bias=mlpb1)
            ps_m2 = ps.tile([128, 16], F32, tag="ps")
            nc.tensor.matmul(ps_m2[0:64, :], mlpw2x, r1Tx[:], start=True, stop=True)
            nc.tensor.matmul(ps_m2[64:128, :], mlpw2x, r1Tx[:], start=True, stop=True)
            eaNT = sb.tile([128, 16], BF16)
            nc.scalar.copy(eaNT[:], ps_m2[:])

            # ------------------------------------------------------- GAT 2
            ps_xt = ps.tile([128, 2, 8], BF16, tag="ps")
            nc.tensor.transpose(ps_xt[:, 0, :], x1b[:, 0:128], ident8)
            nc.tensor.transpose(ps_xt[:, 1, :], x1b[:, 128:256], ident8)
            pe_keepwarm_small(2)
            x1T = sb.tile([128, 2, 8], BF16)
            nc.scalar.activation(x1T[:], ps_xt[:], ACT.Relu)

            ps_h2 = ps.tile([8, 64], F32, tag="ps")
            for j in range(2):
                nc.tensor.matmul(ps_h2[:], x1T[:, j, :], g2l[:, j, :],
                                 start=(j == 0), stop=(j == 1))
            ps_att2 = ps.tile([16, 3], F32, tag="ps")
            nc.tensor.matmul(ps_att2[0:16, 2:3], eaNT[0:64, :], v2h[:],
                             start=True, stop=True)
            for j in range(2):
                nc.tensor.matmul(ps_att2[0:8, 0:2], x1T[:, j, :], Wa2[:, j, :],
                                 start=(j == 0), stop=(j == 1))
            pe_keepwarm_small(4)
            att2 = sb.tile([16, 3], BF16)
            nc.vector.tensor_copy(att2[:], ps_att2[:])
            a2 = att2[0:8, 0:2]
            e16 = att2[0:16, 2:3]
            h2sb = sb.tile([8, 64], BF16)
            nc.vector.tensor_copy(h2sb[:], ps_h2[:])

            ps_al2 = ps.tile([24, 1], F32, tag="ps")
            nc.tensor.matmul(ps_al2[:], PsrcT[:], a2[:, 0:1], start=True, stop=False)
            nc.tensor.matmul(ps_al2[:], PdstT[:], a2[:, 1:2], start=False, stop=False)
            nc.tensor.matmul(ps_al2[:], M24T, e16, start=False, stop=True)
            lr2 = sb.tile([24, 1], F32)
            nc.scalar.activation(lr2[:], ps_al2[:], ACT.Prelu, alpha=0.2)
            ex2 = sb.tile([24, 1], F32)
            nc.scalar.activation(ex2[:], lr2[:], ACT.Exp)
            ex2b = sb.tile([24, 1], BF16)
            nc.vector.tensor_copy(ex2b[:], ex2[:])

            ps_sg2 = ps.tile([24, 64], F32, tag="ps")
            nc.tensor.matmul(ps_sg2[:], PsrcT[:], h2sb[:], start=True, stop=True)
            ps_den2 = ps.tile([8, 1], F32, tag="ps")
            nc.tensor.matmul(ps_den2[:], Pdst[:], ex2b[:], start=True, stop=True)
            rden2 = sb.tile([8, 1], F32)
            nc.vector.reciprocal(rden2[:], ps_den2[:])

            wh2 = sb.tile([24, 64], BF16)
            nc.vector.tensor_scalar(wh2[:], ps_sg2[:], ex2[:], None, ALU.mult)
            ps_num2 = ps.tile([8, 64], F32, tag="ps")
            nc.tensor.matmul(ps_num2[:], Pdst[:], wh2[:], start=True, stop=True)

            x2b = sb.tile([8, 64], BF16)
            nc.vector.scalar_tensor_tensor(x2b[:], ps_num2[:], rden2[:], g2b8,
                                           ALU.mult, ALU.add)
            x2 = sb.tile([8, 64], BF16)
            nc.vector.tensor_scalar(x2[:], x2b[:], 0.0, None, ALU.max)

            # ------------------- deconv pool rows via block-diagonal selector
            pe_keepwarm(3)
            ps_xm = ps.tile([64, 1], F32, tag="ps")
            nc.tensor.matmul(ps_xm[:], x2[:], ones8b, start=True, stop=True)

            sel = sb.tile([128, 4], BF16)
            nc.vector.memset(sel[:], 0.0)
            nc.scalar.copy(sel[0:64, 0:1], ps_xm[:])
            eaview = eaNT[:].rearrange("p (n two) -> p n two", two=2)
            nc.vector.tensor_reduce(sel[64:128, 2:3], eaview[64:128, :, 0],
                                    axis=AXL.X, op=ALU.add)
            # sel2 rows 64:68 select the 4 bias rows appended to dst2x
            sel2 = sb.tile([68, 4], BF16)
            nc.vector.memset(sel2[:], 0.0)
            nc.vector.tensor_copy(sel2[64:68, :], eye4)
            nc.vector.tensor_reduce(sel2[0:64, 3:4], eaview[0:64, :, 1],
                                    axis=AXL.X, op=ALU.add)

            cT = sb.tile([4, 640], BF16)
            ps_cTa = pst.tile([4, 320], F32, tag="pst")
            nc.tensor.matmul(ps_cTa[:], sel[:], dst1[:, 0:320], start=True, stop=False)
            nc.tensor.matmul(ps_cTa[:], sel2[:], dst2x[:, 0:320], start=False, stop=True)
            nc.vector.tensor_copy(cT[:, 0:320], ps_cTa[:])
            ps_cTb = pst.tile([4, 320], F32, tag="pst")
            nc.tensor.matmul(ps_cTb[:], sel[:], dst1[:, 320:640], start=True, stop=False)
            nc.tensor.matmul(ps_cTb[:], sel2[:], dst2x[:, 320:640], start=False, stop=True)
            nc.scalar.copy(cT[:, 320:640], ps_cTb[:])

            # ------------------------------------------------------- CNN_2
            # conv1 split by cell halves so each half starts as soon as its
            # cT columns land
            cTv = cT[:].rearrange("p (b l) -> p b l", b=64)
            ps_c1 = pst.tile([32, 64, 8], F32, tag="pst")
            for k in range(3):
                nc.tensor.matmul(ps_c1[:, 0:32, :], c2w1T[:, k, :],
                                 cTv[:, 0:32, k:k + 8],
                                 start=(k == 0), stop=(k == 2))
            for k in range(3):
                nc.tensor.matmul(ps_c1[:, 32:64, :], c2w1T[:, k, :],
                                 cTv[:, 32:64, k:k + 8],
                                 start=(k == 0), stop=(k == 2))
            # maxpool -> bias -> conv2 -> folded linear, pipelined in cell
            # halves so each stage starts when its half of PSUM lands.
            # l1+l2 are linear-linear (no relu between): folded on host into
            # W12 [128,10]; c2b2's contribution is folded into b12.
            pc1v = ps_c1[:].rearrange("p b (l two) -> p b l two", two=2)
            mp = sb.tile([32, 64, 4], BF16)
            ps_c2 = pst.tile([64, 64, 2], F32, tag="pst")
            y2c = sb.tile([64, 64, 2], BF16)
            ps_l2 = pst.tile([10, 64], F32, tag="pst")
            for hi, (h0, h1) in enumerate(((0, 32), (32, 64))):
                nc.vector.tensor_reduce(mp[:, h0:h1, :], pc1v[:, h0:h1, :, :],
                                        axis=AXL.X, op=ALU.max)
                for k in range(3):
                    nc.tensor.matmul(ps_c2[:, h0:h1, :], c2w2T[:, k, :],
                                     mp[:, h0:h1, k:k + 2],
                                     start=(k == 0), stop=(k == 2))
                nc.vector.tensor_copy(y2c[:, h0:h1, :], ps_c2[:, h0:h1, :])
                for l in range(2):
                    nc.tensor.matmul(ps_l2[:, h0:h1], w12T[:, l, :],
                                     y2c[:, h0:h1, l],
                                     start=(l == 0), stop=(l == 1))
            o10 = sb.tile([10, 64], F32)
            nc.vector.tensor_scalar(o10[:], ps_l2[:], b12, 0.0, ALU.add, ALU.max)
            nc.sync.dma_start(out[:], o10[:])

        _go()
    nc.finalize()
    return nc


_NC = None


def _get_nc():
    global _NC
    if _NC is None:
        _NC = _build_nc()
    return _NC


def _pack_inputs(x_feat, x_feat_tmp, edge_attr, c1w1, c1b1, c1w2, c1b2,
                 g1_lin, g1_as, g1_ad, g1_le, g1_ae, g1_b,
                 g2_lin, g2_as, g2_ad, g2_le, g2_ae, g2_b,
                 mlp_w1, mlp_b1, mlp_w2, mlp_b2,
                 d1w, d1b, d2w, d2b, d3w, d3b,
                 c2w1, c2b1, c2w2, c2b2, c2l1w, c2l1b, c2l2w, c2l2b,
                 edge_index):
    f = np.float32

    def fill(shape, off, blocks):
        arr = np.zeros(shape, dtype=f)
        for name, a in blocks.items():
            a = np.asarray(a, dtype=f)
            arr[0:a.shape[0], off[name]:off[name] + a.shape[1]] = a
        return arr

    xfT = np.zeros((512, 8), dtype=f)
    xfT[0:510] = x_feat.T

    tpad = np.zeros((8, 2, 12), dtype=f)
    for i in range(8):
        r = 1 if i % 2 == 0 else 5
        for b in range(2):
            tpad[i, b, 1:11] = x_feat_tmp[r, b * 4 + i // 2]

    m24mat = np.zeros((16, 24), dtype=f)
    m24mat[:, 0:16] = np.eye(16, dtype=f)
    m24mat[:, 16:24] = 1.0 / 16.0

    # host-folded weight-only attention chains
    asadcols = np.zeros((256, 8), dtype=f)
    aecols = np.zeros((256, 4), dtype=f)
    for h in range(4):
        asadcols[h * 64:(h + 1) * 64, h] = g1_as[h]
        asadcols[h * 64:(h + 1) * 64, 4 + h] = g1_ad[h]
        aecols[h * 64:(h + 1) * 64, h] = g1_ae[h]
    Was = np.asarray(g1_lin, dtype=np.float64) @ asadcols.astype(np.float64)  # [512, 8]
    g1ae_w = np.asarray(g1_le, dtype=np.float64) @ aecols.astype(np.float64)  # [128, 4]
    Wa2 = (np.asarray(g2_lin, dtype=np.float64)
           @ np.stack([g2_as[0], g2_ad[0]], 1).astype(np.float64))            # [256, 2]
    v2harr = np.asarray(g2_le, dtype=np.float64) @ np.asarray(g2_ae[0], dtype=np.float64)

    mA0 = fill((128, _oA0["_W"]), _oA0, {
        "w1T": c1w1.transpose(1, 2, 0).reshape(8, 24),
        "TPAD": tpad.reshape(8, 24),
        "w2T": c1w2.transpose(1, 2, 0).reshape(10, 3),
        "M24T": m24mat,
        "ident16": np.eye(16, dtype=f),
        "XT": xfT.reshape(4, 128, 8).transpose(1, 0, 2).reshape(128, 32),
        "c1b1row": c1b1.reshape(1, 8),
        "ones20": np.ones((1, 20), dtype=f),
        "c1b2c": c1b2.reshape(1, 1),
        "ones16": np.ones((1, 16), dtype=f),
    })
    mA = fill((128, _oA["_W"]), _oA, {
        "mlpw1": mlp_w1,
        "Wa2": Wa2.astype(f).reshape(2, 128, 2).transpose(1, 0, 2).reshape(128, 4),
        "mlpw2x": np.concatenate([np.asarray(mlp_w2, dtype=f),
                                  np.asarray(mlp_b2, dtype=f).reshape(1, 64)], 0),
        "v2h": v2harr.astype(f).reshape(64, 1),
        "mlpb1": mlp_b1.reshape(64, 1),
        "g1b8": np.broadcast_to(g1_b.reshape(1, 256), (8, 256)),
        "g2b8": np.broadcast_to(g2_b.reshape(1, 64), (8, 64)),
        "ones8b": np.full((8, 1), 0.125, dtype=f),
    })

    mB1 = fill((128, _oB1["_W"]), _oB1, {
        "G1L": g1_lin.reshape(4, 128, 256).transpose(1, 0, 2).reshape(128, 1024),
        "g2l": g2_lin.reshape(2, 128, 64).transpose(1, 0, 2).reshape(128, 128),
        "eaT": edge_attr.T,
        "g1ae_w": g1ae_w.astype(f),
        "Was": Was[0:512].astype(f).reshape(4, 128, 8).transpose(1, 0, 2).reshape(128, 32),
        "was510": Was[510:511].astype(f),
        "was511": Was[511:512].astype(f),
        "g1t0": g1_lin[510:511],
        "g1t1": g1_lin[511:512],
    })

    # bias4 rows (unscaled; 0.125 pool scales are folded into d2w/d3w)
    b4 = np.zeros((4, 640), dtype=f)
    b4[0] = np.repeat(d1b, 10)
    b4[1] = np.asarray(x_feat_tmp, dtype=f).reshape(640)   # inf rides the bias row
    b4[2] = np.repeat(d2b, 10)
    b4[3] = np.repeat(d3b, 10)
    dst2x = np.concatenate([np.asarray(d3w, dtype=f).reshape(64, 640) * 0.125,
                            b4], 0)

    # final linear pair (no relu between) folded: c -> c @ W12 + b12f
    W12 = (np.asarray(c2l1w, dtype=np.float64) @ np.asarray(c2l2w, dtype=np.float64))
    cw2b = np.einsum('ock,c->o', np.asarray(c2w2, dtype=np.float64),
                     np.asarray(c2b1, dtype=np.float64))   # conv2(c2b1 const)
    b12f = (np.asarray(c2l1b, dtype=np.float64) @ np.asarray(c2l2w, dtype=np.float64)
            + np.asarray(c2l2b, dtype=np.float64)
            + np.repeat(np.asarray(c2b2, dtype=np.float64), 2) @ W12
            + np.repeat(cw2b, 2) @ W12)
    W12 = W12.astype(f)

    mB2 = fill((128, _oB2["_W"]), _oB2, {
        "dst1": np.concatenate([d1w.reshape(64, 640),
                                d2w.reshape(64, 640) * 0.125], 0),
        "dst2x": dst2x,
        "w12T": np.concatenate([W12[0::2], W12[1::2]], 1),
        "c2w2T": c2w2.transpose(1, 2, 0).reshape(32, 192),
        "c2w1T": c2w1.transpose(1, 2, 0).reshape(4, 96),
    })

    mfarr = fill((32, _oF["_W"]), _oF, {
        "iota_row24": np.broadcast_to(np.arange(8, dtype=f), (24, 8)),
        "iota8": np.arange(8, dtype=f).reshape(8, 1),
        "b12": b12f.astype(f).reshape(10, 1),
        "eye4": np.eye(4, dtype=f),
        "c2b1": c2b1.reshape(32, 1),
    })
    ipack = np.zeros((24, 50), dtype=np.int32)
    blk = np.zeros((8, 2, 24), dtype=np.int32)
    blk[:, :, 0:16] = edge_index[None, :, :]
    blk[:, :, 16:24] = np.arange(8, dtype=np.int32)[None, None, :]
    ipack[0:8, 0:48] = blk.reshape(8, 48)
    ipack[0:16, 48] = edge_index[1]
    ipack[16:24, 48] = np.arange(8, dtype=np.int32)
    mfarr[0:24, _oF["ipackbits"]:_oF["ipackbits"] + 50] = ipack.view(np.float32)

    # f32 block bit-packed into bf16 columns right after mA0
    fblk = np.zeros((128, 2 * _oF["_W"]), dtype=ml_dtypes.bfloat16)
    fblk[0:32] = mfarr.view(np.uint16).view(ml_dtypes.bfloat16)
    mb = np.concatenate(
        [mA0.astype(ml_dtypes.bfloat16), fblk] +
        [m.astype(ml_dtypes.bfloat16) for m in (mA, mB1, mB2)], 1)
    return mb


def _make_ins(inputs):
    mb = _pack_inputs(**inputs)
    return {"mb": mb}


def kernel(**inputs):
    inputs = {k: np.ascontiguousarray(v) for k, v in inputs.items()}
    ins = _make_ins(inputs)
    nc = _get_nc()
    res = run_bass_kernel_spmd(nc, [ins] * 8, core_ids=list(range(8)))
    return np.ascontiguousarray(res.results[0]["out"].T).reshape(8, 8, 10)
